# revision 1
# baseline (speedup 1.0000x reference)
"""Multi-head self-attention (B=2, T=2048, C=1024, H=16) on 8 trn2 cores.

Sharding: core c -> batch b = c//4, heads 4*(c%4) .. 4*(c%4)+3.
Each core: QKV projection for its 4 heads, causal attention in S^T layout
(keys on partitions), partial output projection over its heads' rows of Wo.
Host sums the 4 partials per batch element and adds bo.

All matmuls run in float32r (fp32 with 12-bit mantissa, full PE rate).
"""
import sys

sys.path.insert(0, "/opt/trn_rl_repo")

import numpy as np

B, T, C, H = 2, 2048, 1024, 16
HD = C // H            # 64
NCORES = 8
HPC = H // (NCORES // B)   # heads per core = 4
QB = 128               # q block (columns of S^T)
KB = 128               # k chunk (partitions of S^T)
NJ = T // KB           # 16
NI = T // QB           # 16
SLAB = 512             # q columns processed per attention pass
NSLAB = T // SLAB      # 4
BPS = SLAB // QB       # q blocks per slab = 4
CI = C // 128          # 8 contraction chunks for projections
SCALE = HD ** -0.5

_cache = {}


def _round_fp32r(x: np.ndarray) -> np.ndarray:
    u = np.ascontiguousarray(x, dtype=np.float32).view(np.uint32)
    r = (u + 0x7FF + ((u >> 12) & 1)) & np.uint32(0xFFFFF000)
    return r.view(np.float32)


def _build_plan(mask_bool: np.ndarray):
    """mask_bool: [T, T] (q, k). Returns per (j, i) block types and tiles.

    type 0 = all valid (no mask work), 1 = all masked (skip), 2 = mixed.
    Tiles are stored transposed to match S^T ([k_local, q_local])."""
    btype = np.zeros((NJ, NI), dtype=np.int32)
    tidx = np.full((NJ, NI), -1, dtype=np.int32)
    tiles = []
    tile_map = {}
    for j in range(NJ):
        for i in range(NI):
            sub = mask_bool[i * QB:(i + 1) * QB, j * KB:(j + 1) * KB]
            if sub.all():
                btype[j, i] = 0
            elif not sub.any():
                btype[j, i] = 1
            else:
                btype[j, i] = 2
                key = sub.tobytes()
                if key not in tile_map:
                    tile_map[key] = len(tiles)
                    tiles.append(sub.T.astype(np.float32))
                tidx[j, i] = tile_map[key]
    if not tiles:
        tiles.append(np.ones((KB, QB), dtype=np.float32))
    return btype, tidx, np.stack(tiles)


def _build_program(btype, tidx, n_tiles, apply_qk_bias, apply_v_bias):
    import concourse.bass as bass
    import concourse.tile as tile
    import concourse.mybir as mybir
    from concourse import bacc

    F32 = mybir.dt.float32
    F32R = mybir.dt.float32r
    AF = mybir.ActivationFunctionType
    MULT = mybir.AluOpType.mult

    nc = bacc.Bacc("TRN2", target_bir_lowering=False, debug=False)
    xt_d = nc.dram_tensor("xt", [C, T], F32R, kind="ExternalInput").ap()
    wqk_d = nc.dram_tensor("wqk", [C, 4 * 128], F32R, kind="ExternalInput").ap()
    wv_d = nc.dram_tensor("wv", [C, HPC * HD], F32R, kind="ExternalInput").ap()
    wo_d = nc.dram_tensor("wo", [HPC * HD, C], F32R, kind="ExternalInput").ap()
    mask_d = nc.dram_tensor("masks", [n_tiles, KB, QB], F32,
                            kind="ExternalInput").ap()
    bqk_d = nc.dram_tensor("bqk", [128, 4], F32, kind="ExternalInput").ap()
    bv_d = nc.dram_tensor("bv", [128, 2], F32, kind="ExternalInput").ap()
    zero_d = nc.dram_tensor("zeros", [64, T], F32R, kind="ExternalInput").ap()
    out_d = nc.dram_tensor("out", [T, C], F32, kind="ExternalOutput").ap()

    with tile.TileContext(nc) as tc:
        with tc.tile_pool(name="weights", bufs=1) as wpool, \
             tc.tile_pool(name="acts", bufs=1) as apool:
            # ---- resident SBUF tensors ----
            wo = wpool.tile([128, 2, C], F32R)            # head-pair chunks
            masks = wpool.tile([128, n_tiles * QB], F32)
            bqk = wpool.tile([128, 4], F32)
            bv = wpool.tile([128, 2], F32)
            # q tiles hold (q_hA | q_hB) on partitions 0-63 / 64-127.
            # k is stored zero-padded per head (other head's partitions are
            # zero) so S matmuls present K=128 to the PE — K=64 matmuls do
            # not register as HAM activity and leave the clock at 1.2 GHz.
            qp = [apool.tile([128, T], F32R, tag=f"qp{i}", name=f"qp{i}")
                  for i in range(2)]
            kz = [apool.tile([128, T], F32R, tag=f"kz{i}", name=f"kz{i}")
                  for i in range(4)]          # index = 2*pair + head
            vaug = apool.tile([128, NJ, HPC * (HD + 1)], F32R)


            # ---- QKV projection ----
            # q/k in transposed layout: psum[c_out_pair, t] = W^T x^T
            with tc.tile_pool(name="xtp", bufs=1) as xtp, \
                 tc.tile_pool(name="pproj", bufs=4, space="PSUM") as pp:
                xt = xtp.tile([128, CI, T], F32R)      # x^T, c_in chunked
                wqk = xtp.tile([128, CI, 512], F32R)
                wv = xtp.tile([128, CI, HPC * HD], F32R)
                for ci in range(CI):
                    qt4 = T // 4
                    for qn in range(4):
                        nc.sync.dma_start(
                            xt[:, ci, qn * qt4:(qn + 1) * qt4],
                            xt_d[ci * 128:(ci + 1) * 128,
                                 qn * qt4:(qn + 1) * qt4])
                    nc.sync.dma_start(wqk[:, ci, :],
                                      wqk_d[ci * 128:(ci + 1) * 128, :])
                    nc.sync.dma_start(wv[:, ci, :],
                                      wv_d[ci * 128:(ci + 1) * 128, :])
                # lower-priority loads and inits (after the projection inputs)
                nc.sync.dma_start(wo[:, 0, :], wo_d[0:128, :])
                nc.sync.dma_start(wo[:, 1, :], wo_d[128:256, :])
                for t in range(n_tiles):
                    nc.sync.dma_start(masks[:, t * QB:(t + 1) * QB], mask_d[t])
                nc.sync.dma_start(bqk[:], bqk_d)
                nc.sync.dma_start(bv[:], bv_d)
                va = vaug[:].rearrange("p j (h d) -> p j h d", h=HPC)
                nc.vector.tensor_copy(
                    va[:, :, :, HD:HD + 1],
                    nc.const_aps.tensor(1.0, (128, NJ, HPC, 1)))
                for p in range(2):
                    nc.sync.dma_start(kz[2 * p][64:128, :], zero_d)
                    nc.sync.dma_start(kz[2 * p + 1][0:64, :], zero_d)
                for co in (0, 1, "v", 2, 3):
                    if co == "v":
                        # v projection between the two pairs' qk so pair 0's
                        # attention (needs qk 0/1 + V) can start at full
                        # stride while pair 1's qk still projects
                        for tj in range(NJ):
                            psv = pp.tile([128, HPC * HD], F32, tag="pv",
                                          name="psv")
                            for ci in range(CI):
                                nc.tensor.matmul(
                                    psv[:],
                                    xt[:, ci, tj * 128:(tj + 1) * 128],
                                    wv[:, ci, :],
                                    start=(ci == 0), stop=(ci == CI - 1))
                            nc.vector.tensor_copy(
                                va[:, tj, :, 0:HD],
                                psv[:].rearrange("p (h d) -> p h d", h=HPC))
                        continue
                    pair, is_k = co // 2, co % 2
                    for ts in range(T // 512):
                        sl = slice(ts * 512, (ts + 1) * 512)
                        ps = pp.tile([128, 512], F32, tag="pqk")
                        for ci in range(CI):
                            nc.tensor.matmul(
                                ps[:],
                                wqk[:, ci, co * 128:(co + 1) * 128],
                                xt[:, ci, sl],
                                start=(ci == 0), stop=(ci == CI - 1))
                        if is_k:
                            dsts = [(kz[2 * pair][0:64, sl], ps[0:64, :],
                                     bqk[0:64, co:co + 1]),
                                    (kz[2 * pair + 1][64:128, sl],
                                     ps[64:128, :], bqk[64:128, co:co + 1])]
                        else:
                            dsts = [(qp[pair][:, sl], ps[:],
                                     bqk[:, co:co + 1])]
                        for dst_ap, src_ap, b_ap in dsts:
                            if apply_qk_bias:
                                nc.scalar.activation(dst_ap, src_ap,
                                                     AF.Identity, bias=b_ap,
                                                     scale=1.0)
                            else:
                                nc.vector.tensor_copy(dst_ap, src_ap)

            # ---- attention (S^T layout) ----
            # Chunk pairs share one 2-bank PSUM tile per head so exp covers
            # both in a single ACTIVATE. PV accumulators are per (pair, head)
            # so one pair's softmax division overlaps the other pair's
            # matmuls, keeping the PE continuously busy (HAM stays warm).
            with tc.tile_pool(name="attnp", bufs=1) as attnp:
              attn = [attnp.tile([128, T], F32R, tag=f"attn{p}",
                                 name=f"attn{p}") for p in range(2)]
              with tc.tile_pool(name="psattn", bufs=1, space="PSUM") as sp, \
                   tc.tile_pool(name="psout", bufs=1, space="PSUM") as op, \
                   tc.tile_pool(name="ptp", bufs=5) as ptp, \
                   tc.tile_pool(name="divp", bufs=2) as divp:
                  sums_all = [divp.tile([1, T], F32, tag=f"sums{r}",
                                        name=f"sums{r}", bufs=1)
                              for r in range(4)]
                  pending = []
                  for pair in range(2):
                      q_t = qp[pair]
                      for s in range(NSLAB):
                          if pair == 1 and s >= 1 and pending:
                              pr, hh_ = pending.pop(0)
                              _division_hl(pr, hh_)
                          i_lo, i_hi = s * BPS, (s + 1) * BPS
                          # chunk runs for this slab (shared by all heads)
                          chunks = []
                          for j in range(NJ):
                              live = [i for i in range(i_lo, i_hi)
                                      if btype[j, i] != 1]
                              if live:
                                  chunks.append((j, min(live), max(live)))
                          out_ps = [op.tile([HD + 1, SLAB], F32,
                                            tag=f"outps{_hl}",
                                            name=f"outps{_hl}", bufs=2)
                                    for _hl in range(2)]
                          written = np.zeros(BPS, dtype=bool)
                          for cn, (j, i0, i1) in enumerate(chunks):
                              n_cols = (i1 - i0 + 1) * QB
                              r0 = i0 - i_lo
                              # S^T for both heads into the two banks of one
                              # psum tile; one exp and one mask op cover both
                              sps = sp.tile([128, 2, SLAB], F32,
                                            tag="sst", name="sst", bufs=2)
                              for hl in range(2):
                                  nc.tensor.matmul(
                                      sps[:, hl, 0:n_cols],
                                      kz[2 * pair + hl][:, j * KB:(j + 1) * KB],
                                      q_t[:, i0 * QB:i0 * QB + n_cols],
                                      start=True, stop=True)
                              pt = ptp.tile([128, 2, SLAB], F32R, tag="pt",
                                            name="pt")
                              nc.scalar.activation(pt[:, :, 0:n_cols],
                                                   sps[:, :, 0:n_cols],
                                                   AF.Exp, scale=SCALE)
                              for i in range(i0, i1 + 1):
                                  rel = (i - i0) * QB
                                  if btype[j, i] == 2:
                                      ti = tidx[j, i]
                                      m2 = masks[:, ti * QB:(ti + 1) * QB] \
                                          .unsqueeze(1).broadcast_to(
                                              [128, 2, QB])
                                      nc.gpsimd.tensor_tensor(
                                          out=pt[:, :, rel:rel + QB],
                                          in0=pt[:, :, rel:rel + QB],
                                          in1=m2, op=MULT)
                                  elif btype[j, i] == 1:
                                      nc.gpsimd.memset(pt[:, :, rel:rel + QB],
                                                       0.0)
                              # PV accumulation (runs are <= 512 so no bank
                              # crossing; split only on first-write transitions)
                              segs = []
                              c = r0 * QB
                              end = (i1 - i_lo + 1) * QB
                              while c < end:
                                  st = written[c // QB]
                                  cc = c + QB
                                  while cc < end and written[cc // QB] == st:
                                      cc += QB
                                  segs.append((c, cc, not st))
                                  c = cc
                              last = cn == len(chunks) - 1
                              for hl in range(2):
                                  hh = 2 * pair + hl
                                  for (c0, c1, st_flag) in segs:
                                      nc.tensor.matmul(
                                          out_ps[hl][:, c0:c1],
                                          vaug[:, j, hh * (HD + 1):
                                               (hh + 1) * (HD + 1)],
                                          pt[:, hl, c0 - r0 * QB:c1 - r0 * QB],
                                          start=st_flag, stop=last,
                                          skip_group_check=True)
                              for rr in range(r0, i1 - i_lo + 1):
                                  written[rr] = True
                          # stash undivided PV output + denominators; the
                          # division happens once per pair (keeps the slab
                          # boundary free of cross-engine chains)
                          for hl in range(2):
                              row = 2 * pair + hl
                              dst = attn[pair][64 * hl:64 * hl + 64,
                                               s * SLAB:(s + 1) * SLAB]
                              nc.vector.tensor_copy(dst, out_ps[hl][0:HD, :])
                              nc.vector.tensor_copy(
                                  sums_all[row][0:1,
                                                s * SLAB:(s + 1) * SLAB],
                                  out_ps[hl][HD:HD + 1, :])
                      # consolidated softmax division for this pair —
                      # pair 0's is deferred into pair 1's emission so its
                      # gpsimd broadcast never blocks pair 1's mask ops
                      def _division_hl(pair, hl):
                          row = 2 * pair + hl
                          rec1 = divp.tile([1, T], F32, tag="rec1",
                                           name="rec1")
                          rec128 = divp.tile([128, T], F32, tag="rec128",
                                             name="rec128")
                          nc.vector.reciprocal_approx_fast(
                              rec1[:], sums_all[row][:])
                          nc.gpsimd.partition_broadcast(rec128[:], rec1[:])
                          dst = attn[pair][64 * hl:64 * hl + 64, :]
                          nc.vector.tensor_tensor(
                              out=dst, in0=dst,
                              in1=rec128[64 * hl:64 * hl + 64, :], op=MULT)
                          if apply_v_bias:
                              nc.vector.tensor_scalar(
                                  out=dst, in0=dst,
                                  scalar1=bv[64 * hl:64 * hl + 64,
                                             pair:pair + 1],
                                  scalar2=None, op0=mybir.AluOpType.add)
                      for _hl in range(2):
                          pending.append((pair, _hl))
                      if pair == 1:
                          for pr, hh_ in pending:
                              _division_hl(pr, hh_)
                          pending = []

              # ---- output projection (partial; host sums over cores) ----
              with tc.tile_pool(name="psop", bufs=3, space="PSUM") as opp, \
                   tc.tile_pool(name="osb2", bufs=3) as osb2:
                  for ts in range(NI):
                      ps = opp.tile([128, C], F32, tag="opps", name="opps")
                      for pair in range(2):
                          for n0 in range(0, C, 512):
                              nc.tensor.matmul(
                                  ps[:, n0:n0 + 512],
                                  attn[pair][:, ts * 128:(ts + 1) * 128],
                                  wo[:, pair, n0:n0 + 512],
                                  start=(pair == 0), stop=(pair == 1))
                      ot = osb2.tile([128, C], F32, tag="ot", name="ot")
                      nc.vector.tensor_copy(ot[:], ps[:])
                      nc.sync.dma_start(out_d[ts * 128:(ts + 1) * 128, :],
                                        ot[:])

    nc.compile()
    return nc


def _get_program(mask_bool, apply_qk_bias, apply_v_bias):
    key = (mask_bool.tobytes(), apply_qk_bias, apply_v_bias)
    if key not in _cache:
        btype, tidx, tiles = _build_plan(mask_bool)
        nc = _build_program(btype, tidx, len(tiles), apply_qk_bias,
                            apply_v_bias)
        _cache[key] = (nc, tiles)
    return _cache[key]


def kernel(x, attention_mask, Wqkv, bqkv, Wo, bo, _trace=False):
    from concourse.bass_utils import run_bass_kernel_spmd

    x = np.asarray(x, dtype=np.float32)
    mask_bool = np.asarray(attention_mask)[0, 0] != 0
    Wqkv = np.asarray(Wqkv, dtype=np.float32)
    bqkv = np.asarray(bqkv, dtype=np.float32)
    Wo = np.asarray(Wo, dtype=np.float32)
    bo = np.asarray(bo, dtype=np.float32)

    apply_qk_bias = bool(np.any(bqkv[:2 * C]))
    apply_v_bias = bool(np.any(bqkv[2 * C:]))
    nc, tiles = _get_program(mask_bool, apply_qk_bias, apply_v_bias)

    xts = [_round_fp32r(x[b].T) for b in range(B)]
    zeros = np.zeros((64, T), dtype=np.float32)
    in_maps = []
    for c in range(NCORES):
        b, g = divmod(c, NCORES // B)
        hs = [HPC * g + i for i in range(HPC)]
        # wqk column chunks: [q_h0|q_h1, k_h0|k_h1, q_h2|q_h3, k_h2|k_h3]
        cols, bias_cols = [], []
        for pair in range(2):
            ha, hb = hs[2 * pair], hs[2 * pair + 1]
            for base in (0, C):  # q then k offset in Wqkv columns
                cols.append(Wqkv[:, base + ha * HD:base + (ha + 1) * HD])
                cols.append(Wqkv[:, base + hb * HD:base + (hb + 1) * HD])
                bias_cols.append(np.concatenate([
                    bqkv[base + ha * HD:base + (ha + 1) * HD],
                    bqkv[base + hb * HD:base + (hb + 1) * HD]]))
        wqk_c = _round_fp32r(np.concatenate(cols, axis=1))
        bqk_c = np.stack(bias_cols, axis=1).astype(np.float32)
        wv_c = _round_fp32r(np.concatenate(
            [Wqkv[:, 2 * C + h * HD:2 * C + (h + 1) * HD] for h in hs], axis=1))
        wo_c = _round_fp32r(np.concatenate(
            [Wo[h * HD:(h + 1) * HD, :] for h in hs], axis=0))
        bv_c = np.zeros((128, 2), dtype=np.float32)
        for pair in range(2):
            ha, hb = hs[2 * pair], hs[2 * pair + 1]
            bv_c[0:HD, pair] = bqkv[2 * C + ha * HD:2 * C + (ha + 1) * HD]
            bv_c[HD:128, pair] = bqkv[2 * C + hb * HD:2 * C + (hb + 1) * HD]
        in_maps.append({
            "xt": xts[b], "wqk": wqk_c, "wv": wv_c, "wo": wo_c,
            "masks": tiles, "bqk": bqk_c, "bv": bv_c, "zeros": zeros,
        })

    kwargs = {}
    if _trace:
        kwargs = dict(trace=True, trace_cores=[0])
    res = run_bass_kernel_spmd(nc, in_maps, core_ids=list(range(NCORES)),
                               **kwargs)
    out = np.empty((B, T, C), dtype=np.float32)
    gpb = NCORES // B
    for b in range(B):
        acc = res.results[b * gpb]["out"].astype(np.float32)
        for g in range(1, gpb):
            acc = acc + res.results[b * gpb + g]["out"]
        out[b] = acc + bo
    if _trace:
        kernel._last_results = res
    return out



# revision 8
# speedup vs baseline: 1.3936x; 1.3936x over previous
"""Multi-head self-attention (B=2, T=2048, C=1024, H=16) on 8 trn2 cores.

Sharding: core c -> batch b = c//4, heads 4*(c%4) .. 4*(c%4)+3.
Each core: QKV projection for its 4 heads, causal attention in S^T layout
(keys on partitions), partial output projection over its heads' rows of Wo.
Host sums the 4 partials per batch element and adds bo.

All matmul inputs are bf16 (PSUM accumulates fp32).  The schedule keeps the
PE continuously busy: projections for slab s+1 / the other head-pair are
interleaved as "filler" between attention chunks, the softmax division runs
per slab, and the output projection for slab s overlaps pair-1's slab s+1.
"""
import sys

sys.path.insert(0, "/opt/trn_rl_repo")

import numpy as np

B, T, C, H = 2, 2048, 1024, 16
HD = C // H            # 64
NCORES = 8
HPC = H // (NCORES // B)   # heads per core = 4
QB = 128               # q block (columns of S^T)
KB = 128               # k chunk (partitions of S^T)
NJ = T // KB           # 16
NI = T // QB           # 16
SLAB = 512             # q columns processed per attention pass
NSLAB = T // SLAB      # 4
BPS = SLAB // QB       # q blocks per slab = 4
CI = C // 128          # 8 contraction chunks for projections
SCALE = HD ** -0.5

_cache = {}


def _build_plan(mask_bool: np.ndarray):
    """mask_bool: [T, T] (q, k). Returns per (j, i) block types and tiles.

    type 0 = all valid (no mask work), 1 = all masked (skip), 2 = mixed.
    Tiles are stored transposed to match S^T ([k_local, q_local])."""
    btype = np.zeros((NJ, NI), dtype=np.int32)
    tidx = np.full((NJ, NI), -1, dtype=np.int32)
    tiles = []
    tile_map = {}
    for j in range(NJ):
        for i in range(NI):
            sub = mask_bool[i * QB:(i + 1) * QB, j * KB:(j + 1) * KB]
            if sub.all():
                btype[j, i] = 0
            elif not sub.any():
                btype[j, i] = 1
            else:
                btype[j, i] = 2
                key = sub.tobytes()
                if key not in tile_map:
                    tile_map[key] = len(tiles)
                    tiles.append(sub.T.astype(np.float32))
                tidx[j, i] = tile_map[key]
    if not tiles:
        tiles.append(np.ones((KB, QB), dtype=np.float32))
    return btype, tidx, np.stack(tiles)


def _chunk_list(btype, s):
    """Live (j, i0, i1) chunk runs for slab s."""
    i_lo, i_hi = s * BPS, (s + 1) * BPS
    chunks = []
    for j in range(NJ):
        live = [i for i in range(i_lo, i_hi) if btype[j, i] != 1]
        if live:
            chunks.append((j, min(live), max(live)))
    return chunks


def _build_program(btype, tidx, n_tiles, apply_qk_bias, apply_v_bias):
    import concourse.tile as tile
    import concourse.mybir as mybir
    from concourse import bacc

    F32 = mybir.dt.float32
    BF16 = mybir.dt.bfloat16
    AF = mybir.ActivationFunctionType
    MULT = mybir.AluOpType.mult

    nc = bacc.Bacc("TRN2", target_bir_lowering=False, debug=False)
    xt_d = nc.dram_tensor("xt", [C, T], BF16, kind="ExternalInput").ap()
    wqk_d = nc.dram_tensor("wqk", [C, 4 * 128], BF16, kind="ExternalInput").ap()
    wv_d = nc.dram_tensor("wv", [C, HPC * HD], BF16, kind="ExternalInput").ap()
    wo_d = nc.dram_tensor("wo", [HPC * HD, C], BF16, kind="ExternalInput").ap()
    mask_d = nc.dram_tensor("masks", [n_tiles, KB, 2, QB], BF16,
                            kind="ExternalInput").ap()
    bqk_d = nc.dram_tensor("bqk", [128, 4], F32, kind="ExternalInput").ap()
    bv_d = nc.dram_tensor("bv", [128, 2], F32, kind="ExternalInput").ap()
    zero_d = nc.dram_tensor("zeros", [64, T], BF16, kind="ExternalInput").ap()
    out_d = nc.dram_tensor("out", [T, C], BF16, kind="ExternalOutput").ap()

    with tile.TileContext(nc) as tc:
        with tc.tile_pool(name="w", bufs=1) as wpool, \
             tc.tile_pool(name="psS", bufs=2, space="PSUM") as spool, \
             tc.tile_pool(name="psO", bufs=1, space="PSUM") as bpool, \
             tc.tile_pool(name="psJ", bufs=2, space="PSUM") as jpool, \
             tc.tile_pool(name="ptp", bufs=4) as ptp, \
             tc.tile_pool(name="divp", bufs=2) as divp, \
             tc.tile_pool(name="otp", bufs=3) as otp:
            # ---- resident SBUF tensors ----
            xt = wpool.tile([128, CI, T], BF16)        # x^T, c_in chunked
            wqk = wpool.tile([128, CI, 512], BF16)
            wv = wpool.tile([128, CI, HPC * HD], BF16)
            wo = wpool.tile([128, 2, C], BF16)         # head-pair chunks
            masks = wpool.tile([128, n_tiles, 2, QB], BF16)
            bqk = wpool.tile([128, 4], F32)
            bv = wpool.tile([128, 2], F32)
            # q tiles hold (q_hA | q_hB) on partitions 0-63 / 64-127.
            # k is stored zero-padded per head (other head's partitions are
            # zero) so S matmuls present K=128 to the PE.
            qp = [wpool.tile([128, T], BF16, tag=f"qp{i}", name=f"qp{i}")
                  for i in range(2)]
            kz = [wpool.tile([128, T], BF16, tag=f"kz{i}", name=f"kz{i}")
                  for i in range(4)]          # index = 2*pair + head
            vaug = wpool.tile([128, NJ, HPC * (HD + 1)], BF16)
            attn = [wpool.tile([128, T], BF16, tag=f"attn{p}",
                               name=f"attn{p}") for p in range(2)]

            # ---- DMA preload, ordered so the first projection group can
            # start after ~2MB instead of after the full input set ----
            for ci in range(CI):
                nc.sync.dma_start(wqk[:, ci, :],
                                  wqk_d[ci * 128:(ci + 1) * 128, :])
                nc.sync.dma_start(xt[:, ci, 0:512],
                                  xt_d[ci * 128:(ci + 1) * 128, 0:512])
            for p in range(2):
                nc.sync.dma_start(kz[2 * p][64:128, :], zero_d)
                nc.sync.dma_start(kz[2 * p + 1][0:64, :], zero_d)
            va = vaug[:].rearrange("p j (h d) -> p j h d", h=HPC)
            nc.vector.tensor_copy(
                va[:, :, :, HD:HD + 1],
                nc.const_aps.tensor(1.0, (128, NJ, HPC, 1)))
            for ci in range(CI):
                nc.sync.dma_start(wv[:, ci, :],
                                  wv_d[ci * 128:(ci + 1) * 128, :])
            for qn in range(1, 4):
                for ci in range(CI):
                    nc.sync.dma_start(
                        xt[:, ci, qn * 512:(qn + 1) * 512],
                        xt_d[ci * 128:(ci + 1) * 128, qn * 512:(qn + 1) * 512])
            nc.sync.dma_start(wo[:, 0, :], wo_d[0:128, :])
            nc.sync.dma_start(wo[:, 1, :], wo_d[128:256, :])
            for t in range(n_tiles):
                nc.sync.dma_start(masks[:, t, :, :], mask_d[t])
            nc.sync.dma_start(bqk[:], bqk_d)
            nc.sync.dma_start(bv[:], bv_d)

            # ---- emit helpers (dedup'd so "ensure" calls are idempotent) --
            done = set()
            fillers = []

            def emit_qk(co, ts):
                # q/k projection group: 128 output channels x 512 t columns
                key = ("qk", co, ts)
                if key in done:
                    return False
                done.add(key)
                sl = slice(ts * 512, (ts + 1) * 512)
                ps = jpool.tile([128, 512], F32, tag="pj", name="pj")
                for ci in range(CI):
                    nc.tensor.matmul(
                        ps[:], wqk[:, ci, co * 128:(co + 1) * 128],
                        xt[:, ci, sl], start=(ci == 0), stop=(ci == CI - 1))
                pair, is_k = co // 2, co % 2
                if is_k:
                    dsts = [(kz[2 * pair][0:64, sl], ps[0:64, :],
                             bqk[0:64, co:co + 1]),
                            (kz[2 * pair + 1][64:128, sl], ps[64:128, :],
                             bqk[64:128, co:co + 1])]
                else:
                    dsts = [(qp[pair][:, sl], ps[:], bqk[:, co:co + 1])]
                for dst_ap, src_ap, b_ap in dsts:
                    if apply_qk_bias:
                        nc.scalar.activation(dst_ap, src_ap, AF.Identity,
                                             bias=b_ap, scale=1.0)
                    else:
                        nc.vector.tensor_copy(dst_ap, src_ap)
                return True

            def emit_v(tj):
                key = ("v", tj)
                if key in done:
                    return False
                done.add(key)
                ps = jpool.tile([128, 512], F32, tag="pj", name="pj")
                for ci in range(CI):
                    nc.tensor.matmul(
                        ps[:, 0:HPC * HD], xt[:, ci, tj * 128:(tj + 1) * 128],
                        wv[:, ci, :], start=(ci == 0), stop=(ci == CI - 1))
                nc.vector.tensor_copy(
                    va[:, tj, :, 0:HD],
                    ps[:, 0:HPC * HD].rearrange("p (h d) -> p h d", h=HPC))
                return True

            def emit_outproj(ts, nk):
                # partial out rows [ts*128, (ts+1)*128), cols [nk*512, ...)
                ps = jpool.tile([128, 512], F32, tag="pj", name="pj")
                for pair in range(2):
                    nc.tensor.matmul(
                        ps[:], attn[pair][:, ts * 128:(ts + 1) * 128],
                        wo[:, pair, nk * 512:(nk + 1) * 512],
                        start=(pair == 0), stop=(pair == 1))
                ot = otp.tile([128, 512], BF16, tag="ot", name="ot")
                nc.vector.tensor_copy(ot[:], ps[:])
                nc.sync.dma_start(
                    out_d[ts * 128:(ts + 1) * 128, nk * 512:(nk + 1) * 512],
                    ot[:])
                return True

            def poll():
                # pop until one filler emits real work (dedup'd ones no-op)
                while fillers:
                    if fillers.pop(0)():
                        return

            # ---- attention (S^T layout), software-pipelined ----
            for pair in range(2):
                q_t = qp[pair]
                for s in range(NSLAB):
                    chunks = _chunk_list(btype, s)
                    i_lo = s * BPS
                    # force any projections this slab needs (usually already
                    # pulled in as fillers during the previous slab)
                    emit_qk(2 * pair, s)
                    for (j, _, _) in chunks:
                        emit_qk(2 * pair + 1, j // 4)
                        emit_v(j)
                    # queue filler work for the chunk loop
                    if s + 1 < NSLAB:
                        for (j, _, _) in _chunk_list(btype, s + 1):
                            fillers.append(
                                lambda p=pair, jj=j: emit_qk(2 * p + 1,
                                                             jj // 4))
                            fillers.append(lambda jj=j: emit_v(jj))
                        fillers.append(
                            lambda p=pair, ss=s + 1: emit_qk(2 * p, ss))
                    elif pair == 0:
                        for s2 in range(NSLAB):
                            fillers.append(lambda s2=s2: emit_qk(2, s2))
                            fillers.append(lambda s2=s2: emit_qk(3, s2))

                    out_ps = [bpool.tile([HD + 1, SLAB], F32,
                                         tag=f"outps{_hl}",
                                         name=f"outps{_hl}", bufs=1)
                              for _hl in range(2)]
                    written = np.zeros(BPS, dtype=bool)

                    def emit_pv(j, i0, i1, pt, last):
                        r0 = i0 - i_lo
                        segs = []
                        c = r0 * QB
                        end = (i1 - i_lo + 1) * QB
                        while c < end:
                            st = written[c // QB]
                            cc = c + QB
                            while cc < end and written[cc // QB] == st:
                                cc += QB
                            segs.append((c, cc, not st))
                            c = cc
                        for hl in range(2):
                            hh = 2 * pair + hl
                            for (c0, c1, st_flag) in segs:
                                nc.tensor.matmul(
                                    out_ps[hl][:, c0:c1],
                                    vaug[:, j, hh * (HD + 1):
                                         (hh + 1) * (HD + 1)],
                                    pt[:, hl, c0 - r0 * QB:c1 - r0 * QB],
                                    start=st_flag, stop=last,
                                    skip_group_check=True)
                        for rr in range(r0, i1 - i_lo + 1):
                            written[rr] = True

                    prev = None
                    for cn, (j, i0, i1) in enumerate(chunks):
                        n_cols = (i1 - i0 + 1) * QB
                        sps = spool.tile([128, 2, SLAB], F32, tag="sst",
                                         name="sst", bufs=2)
                        for hl in range(2):
                            nc.tensor.matmul(
                                sps[:, hl, 0:n_cols],
                                kz[2 * pair + hl][:, j * KB:(j + 1) * KB],
                                q_t[:, i0 * QB:i0 * QB + n_cols],
                                start=True, stop=True)
                        pt = ptp.tile([128, 2, SLAB], BF16, tag="pt",
                                      name="pt")
                        nc.scalar.activation(pt[:, :, 0:n_cols],
                                             sps[:, :, 0:n_cols],
                                             AF.Exp, scale=SCALE)
                        for i in range(i0, i1 + 1):
                            rel = (i - i0) * QB
                            if btype[j, i] == 2:
                                ti = tidx[j, i]
                                nc.vector.tensor_tensor(
                                    out=pt[:, :, rel:rel + QB],
                                    in0=pt[:, :, rel:rel + QB],
                                    in1=masks[:, ti, :, :], op=MULT)
                            elif btype[j, i] == 1:
                                nc.gpsimd.memset(pt[:, :, rel:rel + QB], 0.0)
                        poll()
                        if prev is not None:
                            emit_pv(*prev, last=False)
                        prev = (j, i0, i1, pt)
                    emit_pv(*prev, last=True)

                    # per-slab softmax division: 1/den from the augmented
                    # ones-row, broadcast to 128 partitions, fused into attn
                    sums = divp.tile([1, 2 * SLAB], F32, tag="sums",
                                     name="sums")
                    for hl in range(2):
                        nc.vector.tensor_copy(
                            sums[:, hl * SLAB:(hl + 1) * SLAB],
                            out_ps[hl][HD:HD + 1, :])
                    for hl in range(2):
                        dst = attn[pair][64 * hl:64 * hl + 64,
                                         s * SLAB:(s + 1) * SLAB]
                        nc.vector.tensor_copy(dst, out_ps[hl][0:HD, :])
                    rec1 = divp.tile([1, 2 * SLAB], F32, tag="rec1",
                                     name="rec1")
                    nc.vector.reciprocal_approx_fast(rec1[:], sums[:])
                    rec128 = divp.tile([128, 2 * SLAB], F32, tag="rec128",
                                       name="rec128")
                    nc.gpsimd.partition_broadcast(rec128[:], rec1[:])
                    for hl in range(2):
                        dst = attn[pair][64 * hl:64 * hl + 64,
                                         s * SLAB:(s + 1) * SLAB]
                        nc.vector.tensor_tensor(
                            out=dst, in0=dst,
                            in1=rec128[64 * hl:64 * hl + 64,
                                       hl * SLAB:(hl + 1) * SLAB], op=MULT)
                        if apply_v_bias:
                            nc.vector.tensor_scalar(
                                out=dst, in0=dst,
                                scalar1=bv[64 * hl:64 * hl + 64,
                                           pair:pair + 1],
                                scalar2=None, op0=mybir.AluOpType.add)

                    # output projection for this slab overlaps the next
                    # slab's attention (pair 1 only; needs both pairs done)
                    if pair == 1:
                        pieces = [(ts, nk)
                                  for ts in range(s * BPS, (s + 1) * BPS)
                                  for nk in range(2)]
                        if s + 1 < NSLAB:
                            for ts, nk in pieces:
                                fillers.append(
                                    lambda ts=ts, nk=nk: emit_outproj(ts, nk))
                        else:
                            for ts, nk in pieces:
                                emit_outproj(ts, nk)

            # drain any unpulled fillers (small slabs / non-causal masks)
            while fillers:
                fillers.pop(0)()

    nc.compile()
    return nc


def _get_program(mask_bool, apply_qk_bias, apply_v_bias):
    key = (mask_bool.tobytes(), apply_qk_bias, apply_v_bias)
    if key not in _cache:
        btype, tidx, tiles = _build_plan(mask_bool)
        nc = _build_program(btype, tidx, len(tiles), apply_qk_bias,
                            apply_v_bias)
        _cache[key] = (nc, tiles)
    return _cache[key]


def kernel(x, attention_mask, Wqkv, bqkv, Wo, bo, _trace=False):
    from concourse.bass_utils import run_bass_kernel_spmd
    import ml_dtypes

    BF = ml_dtypes.bfloat16
    x = np.asarray(x, dtype=np.float32)
    mask_bool = np.asarray(attention_mask)[0, 0] != 0
    Wqkv = np.asarray(Wqkv, dtype=np.float32)
    bqkv = np.asarray(bqkv, dtype=np.float32)
    Wo = np.asarray(Wo, dtype=np.float32)
    bo = np.asarray(bo, dtype=np.float32)

    apply_qk_bias = bool(np.any(bqkv[:2 * C]))
    apply_v_bias = bool(np.any(bqkv[2 * C:]))
    nc, tiles = _get_program(mask_bool, apply_qk_bias, apply_v_bias)

    xts = [np.ascontiguousarray(x[b].T).astype(BF) for b in range(B)]
    zeros = np.zeros((64, T), dtype=BF)
    # masks pre-doubled for the two heads sharing one exp: [n, 128, 2, 128]
    masks_arr = np.ascontiguousarray(
        np.stack([np.stack([t, t], axis=1) for t in tiles])).astype(BF)
    in_maps = []
    for c in range(NCORES):
        b, g = divmod(c, NCORES // B)
        hs = [HPC * g + i for i in range(HPC)]
        # wqk column chunks: [q_h0|q_h1, k_h0|k_h1, q_h2|q_h3, k_h2|k_h3]
        cols, bias_cols = [], []
        for pair in range(2):
            ha, hb = hs[2 * pair], hs[2 * pair + 1]
            for base in (0, C):  # q then k offset in Wqkv columns
                cols.append(Wqkv[:, base + ha * HD:base + (ha + 1) * HD])
                cols.append(Wqkv[:, base + hb * HD:base + (hb + 1) * HD])
                bias_cols.append(np.concatenate([
                    bqkv[base + ha * HD:base + (ha + 1) * HD],
                    bqkv[base + hb * HD:base + (hb + 1) * HD]]))
        wqk_c = np.concatenate(cols, axis=1).astype(BF)
        bqk_c = np.stack(bias_cols, axis=1).astype(np.float32)
        wv_c = np.concatenate(
            [Wqkv[:, 2 * C + h * HD:2 * C + (h + 1) * HD] for h in hs],
            axis=1).astype(BF)
        wo_c = np.concatenate(
            [Wo[h * HD:(h + 1) * HD, :] for h in hs], axis=0).astype(BF)
        bv_c = np.zeros((128, 2), dtype=np.float32)
        for pair in range(2):
            ha, hb = hs[2 * pair], hs[2 * pair + 1]
            bv_c[0:HD, pair] = bqkv[2 * C + ha * HD:2 * C + (ha + 1) * HD]
            bv_c[HD:128, pair] = bqkv[2 * C + hb * HD:2 * C + (hb + 1) * HD]
        in_maps.append({
            "xt": xts[b], "wqk": wqk_c, "wv": wv_c, "wo": wo_c,
            "masks": masks_arr, "bqk": bqk_c, "bv": bv_c, "zeros": zeros,
        })

    kwargs = {}
    if _trace:
        kwargs = dict(trace=True, trace_cores=[0])
    res = run_bass_kernel_spmd(nc, in_maps, core_ids=list(range(NCORES)),
                               **kwargs)
    out = np.empty((B, T, C), dtype=np.float32)
    gpb = NCORES // B
    for b in range(B):
        acc = res.results[b * gpb]["out"].astype(np.float32)
        for g in range(1, gpb):
            acc = acc + res.results[b * gpb + g]["out"].astype(np.float32)
        out[b] = acc + bo
    if _trace:
        kernel._last_results = res
    return out


# revision 15
# speedup vs baseline: 1.4499x; 1.0405x over previous
"""Multi-head self-attention (B=2, T=2048, C=1024, H=16) on 8 trn2 cores.

Sharding: core c -> batch b = c//4, heads 4*(c%4) .. 4*(c%4)+3.
Each core: QKV projection for its 4 heads, causal attention in S^T layout
(keys on partitions), partial output projection over its heads' rows of Wo.
Host sums the 4 partials per batch element and adds bo.

All matmul inputs are bf16 (PSUM accumulates fp32).  The schedule keeps the
PE continuously busy: projections for slab s+1 / the other head-pair are
interleaved as "filler" between attention chunks, the softmax division runs
per slab, and the output projection for slab s overlaps pair-1's slab s+1.
"""
import sys

sys.path.insert(0, "/opt/trn_rl_repo")

import numpy as np

B, T, C, H = 2, 2048, 1024, 16
HD = C // H            # 64
NCORES = 8
HPC = H // (NCORES // B)   # heads per core = 4
QB = 128               # q block (columns of S^T)
KB = 128               # k chunk (partitions of S^T)
NJ = T // KB           # 16
NI = T // QB           # 16
SLAB = 512             # q columns processed per attention pass
NSLAB = T // SLAB      # 4
BPS = SLAB // QB       # q blocks per slab = 4
CI = C // 128          # 8 contraction chunks for projections
SCALE = HD ** -0.5

_cache = {}


def _build_plan(mask_bool: np.ndarray):
    """mask_bool: [T, T] (q, k). Returns per (j, i) block types and tiles.

    type 0 = all valid (no mask work), 1 = all masked (skip), 2 = mixed.
    Tiles are stored transposed to match S^T ([k_local, q_local])."""
    btype = np.zeros((NJ, NI), dtype=np.int32)
    tidx = np.full((NJ, NI), -1, dtype=np.int32)
    tiles = []
    tile_map = {}
    for j in range(NJ):
        for i in range(NI):
            sub = mask_bool[i * QB:(i + 1) * QB, j * KB:(j + 1) * KB]
            if sub.all():
                btype[j, i] = 0
            elif not sub.any():
                btype[j, i] = 1
            else:
                btype[j, i] = 2
                key = sub.tobytes()
                if key not in tile_map:
                    tile_map[key] = len(tiles)
                    tiles.append(sub.T.astype(np.float32))
                tidx[j, i] = tile_map[key]
    if not tiles:
        tiles.append(np.ones((KB, QB), dtype=np.float32))
    return btype, tidx, np.stack(tiles)


def _chunk_list(btype, s):
    """Live (j, i0, i1) chunk runs for slab s."""
    i_lo, i_hi = s * BPS, (s + 1) * BPS
    chunks = []
    for j in range(NJ):
        live = [i for i in range(i_lo, i_hi) if btype[j, i] != 1]
        if live:
            chunks.append((j, min(live), max(live)))
    return chunks


def _build_program(btype, tidx, n_tiles, apply_qk_bias, apply_v_bias):
    import concourse.tile as tile
    import concourse.mybir as mybir
    from concourse import bacc

    F32 = mybir.dt.float32
    BF16 = mybir.dt.bfloat16
    AF = mybir.ActivationFunctionType
    MULT = mybir.AluOpType.mult

    nc = bacc.Bacc("TRN2", target_bir_lowering=False, debug=False)
    xt_d = nc.dram_tensor("xt", [C, T], BF16, kind="ExternalInput").ap()
    wqk_d = nc.dram_tensor("wqk", [C, 4 * 128], BF16, kind="ExternalInput").ap()
    wv_d = nc.dram_tensor("wv", [C, HPC * HD], BF16, kind="ExternalInput").ap()
    wo_d = nc.dram_tensor("wo", [HPC * HD, C], BF16, kind="ExternalInput").ap()
    mask_d = nc.dram_tensor("masks", [n_tiles, KB, 2, QB], BF16,
                            kind="ExternalInput").ap()
    bqk_d = nc.dram_tensor("bqk", [128, 4], F32, kind="ExternalInput").ap()
    bv_d = nc.dram_tensor("bv", [128, 2], F32, kind="ExternalInput").ap()
    out_d = nc.dram_tensor("out", [T, C], BF16, kind="ExternalOutput").ap()

    with tile.TileContext(nc) as tc:
        with tc.tile_pool(name="w", bufs=1) as wpool, \
             tc.tile_pool(name="psS", bufs=2, space="PSUM") as spool, \
             tc.tile_pool(name="psO", bufs=1, space="PSUM") as bpool, \
             tc.tile_pool(name="psJ", bufs=2, space="PSUM") as jpool, \
             tc.tile_pool(name="ptp", bufs=4) as ptp, \
             tc.tile_pool(name="divp", bufs=2) as divp, \
             tc.tile_pool(name="otp", bufs=3) as otp:
            # ---- resident SBUF tensors ----
            xt = wpool.tile([128, CI, T], BF16)        # x^T, c_in chunked
            wqk = wpool.tile([128, CI, 512], BF16)
            wv = wpool.tile([128, CI, HPC * HD], BF16)
            wo = wpool.tile([128, 2, C], BF16)         # head-pair chunks
            masks = wpool.tile([128, n_tiles, 2, QB], BF16)
            bqk = wpool.tile([128, 4], F32)
            bv = wpool.tile([128, 2], F32)
            # q tiles hold (q_hA | q_hB) on partitions 0-63 / 64-127.
            # k is stored zero-padded per head (other head's partitions are
            # zero) so S matmuls present K=128 to the PE.
            qp = [wpool.tile([128, T], BF16, tag=f"qp{i}", name=f"qp{i}")
                  for i in range(2)]
            kz = [wpool.tile([128, T], BF16, tag=f"kz{i}", name=f"kz{i}")
                  for i in range(4)]          # index = 2*pair + head
            vaug = wpool.tile([128, NJ, HPC * (HD + 1)], BF16)
            attn = [wpool.tile([128, T], BF16, tag=f"attn{p}",
                               name=f"attn{p}") for p in range(2)]

            # ---- DMA preload: few big strided transfers, ordered so the
            # first projection group can start after ~1.5MB ----
            xtv = xt_d.rearrange("(ci p) t -> p ci t", p=128)
            wqkv = wqk_d.rearrange("(ci p) n -> p ci n", p=128)
            wvv = wv_d.rearrange("(ci p) n -> p ci n", p=128)
            wov = wo_d.rearrange("(k p) n -> p k n", p=128)
            maskv = mask_d.rearrange("n p h q -> p n h q")
            nc.sync.dma_start(wqk[:, :, 0:256], wqkv[:, :, 0:256])
            nc.sync.dma_start(xt[:, :, 0:512], xtv[:, :, 0:512])
            nc.sync.dma_start(wqk[:, :, 256:512], wqkv[:, :, 256:512])
            # k zero-padding via gpsimd (Pool idle during the lead-in)
            for p in range(2):
                nc.gpsimd.memset(kz[2 * p][64:128, :], 0.0)
                nc.gpsimd.memset(kz[2 * p + 1][0:64, :], 0.0)
            va = vaug[:].rearrange("p j (h d) -> p j h d", h=HPC)
            nc.vector.tensor_copy(
                va[:, :, :, HD:HD + 1],
                nc.const_aps.tensor(1.0, (128, NJ, HPC, 1)))
            nc.sync.dma_start(wv[:], wvv[:])
            nc.sync.dma_start(xt[:, :, 512:1024], xtv[:, :, 512:1024])
            nc.sync.dma_start(xt[:, :, 1024:1536], xtv[:, :, 1024:1536])
            nc.sync.dma_start(xt[:, :, 1536:2048], xtv[:, :, 1536:2048])
            nc.sync.dma_start(wo[:], wov[:])
            nc.sync.dma_start(masks[:], maskv[:])
            nc.sync.dma_start(bqk[:], bqk_d)
            nc.sync.dma_start(bv[:], bv_d)

            # ---- emit helpers (dedup'd so "ensure" calls are idempotent) --
            done = set()
            fillers = []

            def emit_qk(co, ts):
                # q/k projection group: 128 output channels x 512 t columns
                key = ("qk", co, ts)
                if key in done:
                    return False
                done.add(key)
                sl = slice(ts * 512, (ts + 1) * 512)
                ps = jpool.tile([128, 512], F32, tag="pj", name="pj")
                for ci in range(CI):
                    nc.tensor.matmul(
                        ps[:], wqk[:, ci, co * 128:(co + 1) * 128],
                        xt[:, ci, sl], start=(ci == 0), stop=(ci == CI - 1))
                pair, is_k = co // 2, co % 2
                if is_k:
                    dsts = [(kz[2 * pair][0:64, sl], ps[0:64, :],
                             bqk[0:64, co:co + 1]),
                            (kz[2 * pair + 1][64:128, sl], ps[64:128, :],
                             bqk[64:128, co:co + 1])]
                else:
                    dsts = [(qp[pair][:, sl], ps[:], bqk[:, co:co + 1])]
                for dst_ap, src_ap, b_ap in dsts:
                    if apply_qk_bias:
                        nc.scalar.activation(dst_ap, src_ap, AF.Identity,
                                             bias=b_ap, scale=1.0)
                    else:
                        nc.vector.tensor_copy(dst_ap, src_ap)
                return True

            def emit_v(tj):
                key = ("v", tj)
                if key in done:
                    return False
                done.add(key)
                ps = jpool.tile([128, 512], F32, tag="pj", name="pj")
                for ci in range(CI):
                    nc.tensor.matmul(
                        ps[:, 0:HPC * HD], xt[:, ci, tj * 128:(tj + 1) * 128],
                        wv[:, ci, :], start=(ci == 0), stop=(ci == CI - 1))
                nc.vector.tensor_copy(
                    va[:, tj, :, 0:HD],
                    ps[:, 0:HPC * HD].rearrange("p (h d) -> p h d", h=HPC))
                return True

            def emit_outproj(ts, nk):
                # partial out rows [ts*128, (ts+1)*128), cols [nk*512, ...)
                ps = jpool.tile([128, 512], F32, tag="pj", name="pj")
                for pair in range(2):
                    nc.tensor.matmul(
                        ps[:], attn[pair][:, ts * 128:(ts + 1) * 128],
                        wo[:, pair, nk * 512:(nk + 1) * 512],
                        start=(pair == 0), stop=(pair == 1))
                ot = otp.tile([128, 512], BF16, tag="ot", name="ot")
                if nk == 0:
                    # alternate the psum->sbuf cast between DVE and the Act
                    # engine (Act has slack outside the exp stream)
                    nc.vector.tensor_copy(ot[:], ps[:])
                else:
                    nc.scalar.activation(ot[:], ps[:], AF.Copy, 0.0,
                                         scale=1.0)
                nc.sync.dma_start(
                    out_d[ts * 128:(ts + 1) * 128, nk * 512:(nk + 1) * 512],
                    ot[:])
                return True

            def poll():
                # pop until one filler emits real work (dedup'd ones no-op)
                while fillers:
                    if fillers.pop(0)():
                        return

            # ---- attention (S^T layout), software-pipelined ----
            for pair in range(2):
                q_t = qp[pair]
                for s in range(NSLAB):
                    chunks = _chunk_list(btype, s)
                    i_lo = s * BPS
                    # force any projections this slab needs (usually already
                    # pulled in as fillers during the previous slab)
                    emit_qk(2 * pair, s)
                    for (j, _, _) in chunks:
                        emit_qk(2 * pair + 1, j // 4)
                        emit_v(j)
                    # queue filler work for the chunk loop
                    if s + 1 < NSLAB:
                        for (j, _, _) in _chunk_list(btype, s + 1):
                            fillers.append(
                                lambda p=pair, jj=j: emit_qk(2 * p + 1,
                                                             jj // 4))
                            fillers.append(lambda jj=j: emit_v(jj))
                        fillers.append(
                            lambda p=pair, ss=s + 1: emit_qk(2 * p, ss))
                    elif pair == 0:
                        for s2 in range(NSLAB):
                            fillers.append(lambda s2=s2: emit_qk(2, s2))
                            fillers.append(lambda s2=s2: emit_qk(3, s2))

                    out_ps = [bpool.tile([HD + 1, SLAB], F32,
                                         tag=f"outps{_hl}",
                                         name=f"outps{_hl}", bufs=1)
                              for _hl in range(2)]
                    written = np.zeros(BPS, dtype=bool)

                    def emit_pv(j, i0, i1, pt, last):
                        r0 = i0 - i_lo
                        segs = []
                        c = r0 * QB
                        end = (i1 - i_lo + 1) * QB
                        while c < end:
                            st = written[c // QB]
                            cc = c + QB
                            while cc < end and written[cc // QB] == st:
                                cc += QB
                            segs.append((c, cc, not st))
                            c = cc
                        for hl in range(2):
                            hh = 2 * pair + hl
                            for (c0, c1, st_flag) in segs:
                                nc.tensor.matmul(
                                    out_ps[hl][:, c0:c1],
                                    vaug[:, j, hh * (HD + 1):
                                         (hh + 1) * (HD + 1)],
                                    pt[:, hl, c0 - r0 * QB:c1 - r0 * QB],
                                    start=st_flag, stop=last,
                                    skip_group_check=True)
                        for rr in range(r0, i1 - i_lo + 1):
                            written[rr] = True

                    prev = None
                    for cn, (j, i0, i1) in enumerate(chunks):
                        n_cols = (i1 - i0 + 1) * QB
                        sps = spool.tile([128, 2, SLAB], F32, tag="sst",
                                         name="sst", bufs=2)
                        for hl in range(2):
                            nc.tensor.matmul(
                                sps[:, hl, 0:n_cols],
                                kz[2 * pair + hl][:, j * KB:(j + 1) * KB],
                                q_t[:, i0 * QB:i0 * QB + n_cols],
                                start=True, stop=True)
                        pt = ptp.tile([128, 2, SLAB], BF16, tag="pt",
                                      name="pt")
                        nc.scalar.activation(pt[:, :, 0:n_cols],
                                             sps[:, :, 0:n_cols],
                                             AF.Exp, scale=SCALE)
                        for i in range(i0, i1 + 1):
                            rel = (i - i0) * QB
                            if btype[j, i] == 2:
                                ti = tidx[j, i]
                                nc.vector.tensor_tensor(
                                    out=pt[:, :, rel:rel + QB],
                                    in0=pt[:, :, rel:rel + QB],
                                    in1=masks[:, ti, :, :], op=MULT)
                            elif btype[j, i] == 1:
                                nc.gpsimd.memset(pt[:, :, rel:rel + QB], 0.0)
                        poll()
                        if prev is not None:
                            emit_pv(*prev, last=False)
                        prev = (j, i0, i1, pt)
                    emit_pv(*prev, last=True)

                    # per-slab softmax division: 1/den from the augmented
                    # ones-row, broadcast to 128 partitions, applied in attn
                    sums = divp.tile([1, 2 * SLAB], F32, tag="sums",
                                     name="sums")
                    for hl in range(2):
                        nc.vector.tensor_copy(
                            sums[:, hl * SLAB:(hl + 1) * SLAB],
                            out_ps[hl][HD:HD + 1, :])
                    for hl in range(2):
                        dst = attn[pair][64 * hl:64 * hl + 64,
                                         s * SLAB:(s + 1) * SLAB]
                        nc.vector.tensor_copy(dst, out_ps[hl][0:HD, :])
                    rec1 = divp.tile([1, 2 * SLAB], F32, tag="rec1",
                                     name="rec1")
                    nc.vector.reciprocal_approx_fast(rec1[:], sums[:])
                    rec128 = divp.tile([128, 2 * SLAB], F32, tag="rec128",
                                       name="rec128")
                    nc.gpsimd.partition_broadcast(rec128[:], rec1[:])
                    for hl in range(2):
                        dst = attn[pair][64 * hl:64 * hl + 64,
                                         s * SLAB:(s + 1) * SLAB]
                        nc.vector.tensor_tensor(
                            out=dst, in0=dst,
                            in1=rec128[64 * hl:64 * hl + 64,
                                       hl * SLAB:(hl + 1) * SLAB], op=MULT)
                        if apply_v_bias:
                            nc.vector.tensor_scalar(
                                out=dst, in0=dst,
                                scalar1=bv[64 * hl:64 * hl + 64,
                                           pair:pair + 1],
                                scalar2=None, op0=mybir.AluOpType.add)

                    # output projection for this slab overlaps the next
                    # slab's attention (pair 1 only; needs both pairs done)
                    if pair == 1:
                        pieces = [(ts, nk)
                                  for ts in range(s * BPS, (s + 1) * BPS)
                                  for nk in range(2)]
                        if s + 1 < NSLAB:
                            for ts, nk in pieces:
                                fillers.append(
                                    lambda ts=ts, nk=nk: emit_outproj(ts, nk))
                        else:
                            for ts, nk in pieces:
                                emit_outproj(ts, nk)

            # drain any unpulled fillers (small slabs / non-causal masks)
            while fillers:
                fillers.pop(0)()

    nc.compile()
    return nc


def _get_program(mask_bool, apply_qk_bias, apply_v_bias):
    key = (mask_bool.tobytes(), apply_qk_bias, apply_v_bias)
    if key not in _cache:
        btype, tidx, tiles = _build_plan(mask_bool)
        nc = _build_program(btype, tidx, len(tiles), apply_qk_bias,
                            apply_v_bias)
        _cache[key] = (nc, tiles)
    return _cache[key]


def kernel(x, attention_mask, Wqkv, bqkv, Wo, bo, _trace=False):
    from concourse.bass_utils import run_bass_kernel_spmd
    import ml_dtypes

    BF = ml_dtypes.bfloat16
    x = np.asarray(x, dtype=np.float32)
    mask_bool = np.asarray(attention_mask)[0, 0] != 0
    Wqkv = np.asarray(Wqkv, dtype=np.float32)
    bqkv = np.asarray(bqkv, dtype=np.float32)
    Wo = np.asarray(Wo, dtype=np.float32)
    bo = np.asarray(bo, dtype=np.float32)

    apply_qk_bias = bool(np.any(bqkv[:2 * C]))
    apply_v_bias = bool(np.any(bqkv[2 * C:]))
    nc, tiles = _get_program(mask_bool, apply_qk_bias, apply_v_bias)

    xts = [np.ascontiguousarray(x[b].T).astype(BF) for b in range(B)]
    # masks pre-doubled for the two heads sharing one exp: [n, 128, 2, 128]
    masks_arr = np.ascontiguousarray(
        np.stack([np.stack([t, t], axis=1) for t in tiles])).astype(BF)
    in_maps = []
    for c in range(NCORES):
        b, g = divmod(c, NCORES // B)
        hs = [HPC * g + i for i in range(HPC)]
        # wqk column chunks: [q_h0|q_h1, k_h0|k_h1, q_h2|q_h3, k_h2|k_h3]
        cols, bias_cols = [], []
        for pair in range(2):
            ha, hb = hs[2 * pair], hs[2 * pair + 1]
            for base in (0, C):  # q then k offset in Wqkv columns
                cols.append(Wqkv[:, base + ha * HD:base + (ha + 1) * HD])
                cols.append(Wqkv[:, base + hb * HD:base + (hb + 1) * HD])
                bias_cols.append(np.concatenate([
                    bqkv[base + ha * HD:base + (ha + 1) * HD],
                    bqkv[base + hb * HD:base + (hb + 1) * HD]]))
        wqk_c = np.concatenate(cols, axis=1).astype(BF)
        bqk_c = np.stack(bias_cols, axis=1).astype(np.float32)
        wv_c = np.concatenate(
            [Wqkv[:, 2 * C + h * HD:2 * C + (h + 1) * HD] for h in hs],
            axis=1).astype(BF)
        wo_c = np.concatenate(
            [Wo[h * HD:(h + 1) * HD, :] for h in hs], axis=0).astype(BF)
        bv_c = np.zeros((128, 2), dtype=np.float32)
        for pair in range(2):
            ha, hb = hs[2 * pair], hs[2 * pair + 1]
            bv_c[0:HD, pair] = bqkv[2 * C + ha * HD:2 * C + (ha + 1) * HD]
            bv_c[HD:128, pair] = bqkv[2 * C + hb * HD:2 * C + (hb + 1) * HD]
        in_maps.append({
            "xt": xts[b], "wqk": wqk_c, "wv": wv_c, "wo": wo_c,
            "masks": masks_arr, "bqk": bqk_c, "bv": bv_c,
        })

    kwargs = {}
    if _trace:
        kwargs = dict(trace=True, trace_cores=[0])
    res = run_bass_kernel_spmd(nc, in_maps, core_ids=list(range(NCORES)),
                               **kwargs)
    out = np.empty((B, T, C), dtype=np.float32)
    gpb = NCORES // B
    for b in range(B):
        acc = res.results[b * gpb]["out"].astype(np.float32)
        for g in range(1, gpb):
            acc = acc + res.results[b * gpb + g]["out"].astype(np.float32)
        out[b] = acc + bo
    if _trace:
        kernel._last_results = res
    return out
